# revision 19
# baseline (speedup 1.0000x reference)
"""Distributed Trainium2 (Bass/Tile) kernel for single-head latent attention.

Reference computation (B=4, S=4096, D=1024, DL=64):
    qkv = x @ Wd + bd; q,k,v = split(qkv)
    logits = (q @ k^T) / sqrt(DL) / TEMP, key-masked
    out = softmax(logits) @ v @ Wu + bu

Sharding: data-parallel over (batch, seq-half) -> 8 shards of 2048 query rows.
Each core re-computes K/V for its full batch from x^T (no collectives).

Per-core device algorithm (all matmuls in float32r, 1 cycle/row):
  - qkvT = Wd^T @ xT directly in transposed layout [e, s] (lhsT = Wd chunks)
  - logitsT[j, q] per 128-key chunk: lhsT = kT chunk [64,128], rhs = qT
  - expT = ACT Exp(1.25 * logitsT + maskbias[j])   (maskbias = -40 or -1e30;
    no row-max subtraction needed: scaled logits are bounded ~(-95, 95))
  - ctxU/Z accumulate in PSUM over all key chunks via augmented PV matmul:
    lhsT = [ones | v] [128, 65] -> row 0 = Z, rows 1:65 = ctxU  (plain sums,
    so the flash merge over key chunks is just PSUM accumulation)
  - normalize: ctxn = ctxU * broadcast(1/Z)  (broadcast via K=1 ones matmul);
    row 0 becomes exactly 1.0
  - out = ctxn^T @ [bu; Wu]  (bias-add folded into the matmul via the 1-row)
"""

import sys

if "/opt/trn_rl_repo" not in sys.path:
    sys.path.insert(0, "/opt/trn_rl_repo")

import numpy as np

from concourse import bacc, bass, tile
from concourse import mybir
from concourse.masks import make_identity

F32 = mybir.dt.float32
F32R = mybir.dt.float32r
BF16 = mybir.dt.bfloat16

# x / Wd in bf16 halves the dominant DMA traffic (x is only used for the
# qkv projection; logit error stays ~3e-2 absolute, well inside the gate).
USE_BF16_X = False

B, S, D, DL = 4, 4096, 1024, 64
N_CORES = 8
S_LOC = S // 2          # 2048 query rows per core
SR = 512                # projection s-range width
NR_FULL = S // SR       # 8
JC = 128                # key chunk
NJ = S // JC            # 32
QH = 1024               # exp/logits q-half width
SCALE = 1.25            # 1/sqrt(64)/0.1
LOGIT_SHIFT = -40.0
MASKED_BIAS = -1e30

_CACHE = {}


def r32(ap):
    return ap.bitcast(F32R)


def build_graph():
    """Build the (core-agnostic) Bacc graph. Each core's xT/mask are rotated
    host-side so its local query half always sits in columns 0:2048."""
    half = 0
    nc = bacc.Bacc("TRN2", target_bir_lowering=False, debug=False,
                   num_devices=N_CORES)

    XDT = BF16 if USE_BF16_X else F32R
    xT_d = nc.dram_tensor("xT", [D, S], XDT, kind="ExternalInput").ap()
    wd_d = nc.dram_tensor("Wd", [D, 3 * DL], XDT, kind="ExternalInput").ap()
    wub_d = nc.dram_tensor("Wub", [DL + 1, D], F32R, kind="ExternalInput").ap()
    bdq_d = nc.dram_tensor("bd_q", [64, 1], F32, kind="ExternalInput").ap()
    bdk_d = nc.dram_tensor("bd_k", [64, 1], F32, kind="ExternalInput").ap()
    bdv_d = nc.dram_tensor("bd_v", [64, 1], F32, kind="ExternalInput").ap()
    mb_d = nc.dram_tensor("maskbias", [128, NJ], F32, kind="ExternalInput").ap()
    out_d = nc.dram_tensor("out", [S_LOC, D], F32, kind="ExternalOutput").ap()

    nloc = S_LOC // SR                  # 4 local s-ranges (always ranges 0:4)

    with tile.TileContext(nc) as tc, nc.allow_low_precision(
            reason="float32r (tf32-like) tiles feed full-rate PE matmuls; "
                   "~10-bit mantissa is far inside the 2e-2 error budget"):
        with (
            tc.tile_pool(name="consts", bufs=1) as consts,
            tc.tile_pool(name="acts", bufs=1) as acts,
            tc.tile_pool(name="xp", bufs=2) as xp,
            tc.tile_pool(name="ep", bufs=4) as ep,
        ):
            # ---- constants -------------------------------------------------
            wd_s = consts.tile([128, 8 * 192], XDT)
            for k in range(8):
                nc.sync.dma_start(out=wd_s[:, k * 192:(k + 1) * 192],
                                  in_=wd_d[k * 128:(k + 1) * 128, :])
            wub_s = consts.tile([DL + 1, D], F32R)
            nc.sync.dma_start(out=wub_s[:], in_=wub_d[:])
            bdq_s = consts.tile([64, 1], F32)
            nc.sync.dma_start(out=bdq_s[:], in_=bdq_d[:])
            bdk_s = consts.tile([64, 1], F32)
            nc.sync.dma_start(out=bdk_s[:], in_=bdk_d[:])
            bdv_s = consts.tile([64, 1], F32)
            nc.sync.dma_start(out=bdv_s[:], in_=bdv_d[:])
            mb_s = consts.tile([128, NJ], F32)
            nc.sync.dma_start(out=mb_s[:], in_=mb_d[:])
            ident = consts.tile([64, 64], F32)
            make_identity(nc, ident[:])
            ones_col = consts.tile([1, 128], F32)
            nc.vector.memset(ones_col[:], 1.0)
            ones_stage = consts.tile([128, NJ], F32)
            nc.vector.memset(ones_stage[:], 1.0)

            # ---- activations (SBUF-resident) -------------------------------
            qT_s = acts.tile([64, S_LOC], F32R)
            kT_s = acts.tile([64, S], F32R)
            vT_s = acts.tile([64, S], F32)
            # PV stationary: col 0 = ones, cols 1:65 = v rows; per key chunk
            v_aug = acts.tile([128, NJ * 65], F32R)
            # ones column (stride-65 view) via f32->f32r rounding copy
            v_aug_ones = v_aug[:].rearrange("p (c w) -> p c w", w=65)[:, :, 0]
            nc.vector.tensor_copy(v_aug_ones, ones_stage[:])
            ctxu_s = acts.tile([DL + 1, S_LOC], F32)
            rzb_s = acts.tile([DL + 1, S_LOC], F32)
            ctxn_s = acts.tile([DL + 1, S_LOC], F32R)

            # ---- phase 1: qkv projection ----------------------------------
            with tc.tile_pool(name="pp", bufs=4, space="PSUM") as pp:
                for r in range(NR_FULL):
                    local = r < nloc
                    xt = xp.tile([128, 8 * SR], XDT, tag="xt")
                    for k in range(8):
                        nc.sync.dma_start(
                            out=xt[:, k * SR:(k + 1) * SR],
                            in_=xT_d[k * 128:(k + 1) * 128, r * SR:(r + 1) * SR])
                    col = slice(r * SR, (r + 1) * SR)
                    pieces = [(1, kT_s, bdk_s), (2, vT_s, bdv_s)]
                    if local:
                        pieces = [(0, qT_s, bdq_s)] + pieces
                    for piece, dst, bias in pieces:
                        ps = pp.tile([64, SR], F32, tag="p", name=f"ps{r}_{piece}")
                        for k in range(8):
                            nc.tensor.matmul(
                                ps[:],
                                wd_s[:, k * 192 + piece * 64:
                                     k * 192 + piece * 64 + 64],
                                xt[:, k * SR:(k + 1) * SR],
                                start=(k == 0), stop=(k == 7))
                        nc.vector.tensor_scalar_add(dst[:, col], ps[:], bias[:])

            # ---- phase 1b: transpose v into v_aug --------------------------
            with tc.tile_pool(name="pt", bufs=2, space="PSUM") as pt:
                for c in range(NJ):
                    vt_ps = pt.tile([128, 64], F32, tag="t")
                    nc.tensor.transpose(vt_ps[:],
                                        vT_s[:, c * JC:(c + 1) * JC],
                                        ident[:])
                    nc.vector.tensor_copy(v_aug[:, c * 65 + 1:(c + 1) * 65],
                                          vt_ps[:])

            # ---- phase 2: attention ---------------------------------------
            with (
                tc.tile_pool(name="pl", bufs=2, space="PSUM") as pl,
                tc.tile_pool(name="pc", bufs=4, space="PSUM") as pc,
            ):
                ctx_ps = [pc.tile([DL + 1, SR], F32, tag="c", name=f"ctx_ps{i}")
                          for i in range(4)]
                for ji, c in enumerate(range(NJ)):
                    kT_c = kT_s[:, c * JC:(c + 1) * JC]
                    for hq in range(2):  # q halves of 1024
                        lg = pl.tile([128, QH], F32, tag="l")
                        for s2 in range(2):
                            qq = hq * QH + s2 * SR
                            nc.tensor.matmul(
                                lg[:, s2 * SR:(s2 + 1) * SR], kT_c,
                                qT_s[:, qq:qq + SR],
                                start=True, stop=True)
                        ex = ep.tile([128, QH], F32R, tag="e")
                        nc.scalar.activation(
                            ex[:], lg[:], mybir.ActivationFunctionType.Exp,
                            bias=mb_s[:, c:c + 1], scale=SCALE)
                        for s2 in range(2):
                            qr = hq * 2 + s2
                            nc.tensor.matmul(
                                ctx_ps[qr][:], v_aug[:, c * 65:(c + 1) * 65],
                                ex[:, s2 * SR:(s2 + 1) * SR],
                                start=(ji == 0), stop=(ji == NJ - 1))
                for qr in range(4):
                    nc.vector.tensor_copy(ctxu_s[:, qr * SR:(qr + 1) * SR],
                                          ctx_ps[qr][:])
                # broadcast 1/Z to all 65 partitions: Zb = ones^T @ Z-row
                for qr in range(4):
                    zb = pl.tile([DL + 1, SR], F32, tag="l")
                    nc.tensor.matmul(zb[:], ones_col[:, 0:DL + 1],
                                     ctxu_s[0:1, qr * SR:(qr + 1) * SR],
                                     start=True, stop=True)
                    nc.vector.reciprocal(rzb_s[:, qr * SR:(qr + 1) * SR], zb[:])
                nc.vector.tensor_mul(ctxn_s[:], ctxu_s[:], rzb_s[:])

            # ---- phase 3: up-projection (bias folded via ctxn row 0 == 1) --
            with (
                tc.tile_pool(name="po", bufs=3, space="PSUM") as po,
                tc.tile_pool(name="ob", bufs=3) as ob,
            ):
                for st in range(S_LOC // 128):
                    up = po.tile([128, D], F32, tag="o")
                    for s2 in range(2):
                        nc.tensor.matmul(
                            up[:, s2 * SR:(s2 + 1) * SR],
                            ctxn_s[:, st * 128:(st + 1) * 128],
                            wub_s[:, s2 * SR:(s2 + 1) * SR],
                            start=True, stop=True)
                    osb = ob.tile([128, D], F32, tag="ot")
                    nc.vector.tensor_copy(osb[:], up[:])
                    nc.sync.dma_start(out=out_d[st * 128:(st + 1) * 128, :],
                                      in_=osb[:])

    nc.compile()
    return nc


def get_graph():
    if "graph" not in _CACHE:
        _CACHE["graph"] = build_graph()
    return _CACHE["graph"]


def make_in_maps(x, attention_mask, Wd, bd, Wu, bu):
    if USE_BF16_X:
        import ml_dtypes
        xdt = np.dtype(ml_dtypes.bfloat16)
    else:
        xdt = np.float32
    wub = np.ascontiguousarray(
        np.concatenate([bu[None, :], Wu], axis=0).astype(np.float32))
    wd_c = np.ascontiguousarray(Wd.astype(xdt))
    bd_q = np.ascontiguousarray(bd[0:64].reshape(64, 1).astype(np.float32))
    bd_k = np.ascontiguousarray(bd[64:128].reshape(64, 1).astype(np.float32))
    bd_v = np.ascontiguousarray(bd[128:192].reshape(64, 1).astype(np.float32))
    in_maps = []
    for c in range(N_CORES):
        b, h = c // 2, c % 2
        xT = x[b].T                                          # [D, S] view
        if h:
            xT = np.concatenate([xT[:, S_LOC:], xT[:, :S_LOC]], axis=1)
        m = attention_mask[b]
        if h:
            m = np.concatenate([m[S_LOC:], m[:S_LOC]])
        mb = np.where(m > 0, np.float32(LOGIT_SHIFT),
                      np.float32(MASKED_BIAS)).astype(np.float32)
        in_maps.append({
            "xT": np.ascontiguousarray(xT).astype(xdt),
            "Wd": wd_c,
            "Wub": wub,
            "bd_q": bd_q,
            "bd_k": bd_k,
            "bd_v": bd_v,
            "maskbias": np.ascontiguousarray(mb.reshape(NJ, 128).T),
        })
    return in_maps


def kernel(x, attention_mask, Wd, bd, Wu, bu):
    from concourse import bass_utils

    x = np.asarray(x, dtype=np.float32)
    attention_mask = np.asarray(attention_mask)
    Wd = np.asarray(Wd, dtype=np.float32)
    bd = np.asarray(bd, dtype=np.float32)
    Wu = np.asarray(Wu, dtype=np.float32)
    bu = np.asarray(bu, dtype=np.float32)

    nc = get_graph()
    in_maps = make_in_maps(x, attention_mask, Wd, bd, Wu, bu)
    res = bass_utils.run_bass_kernel_spmd(nc, in_maps, list(range(N_CORES)))
    out = np.empty((B, S, D), dtype=np.float32)
    for c in range(N_CORES):
        b, h = c // 2, c % 2
        out[b, h * S_LOC:(h + 1) * S_LOC, :] = res.results[c]["out"]
    return out
